# revision 6
# baseline (speedup 1.0000x reference)
"""Trainium2 Bass kernel for a dense transformer block (GPT style).

  out = h + gelu(h @ W1 + b1) @ W2 + b2,  h = x + attn(x)
  attn: 16-head causal self-attention, scale = 1/sqrt(H) (full hidden dim).

Shapes: x [4, 2048, 1024], NH=16, HD=64, FF=4096. 8 NeuronCores.

Distribution:
  - QKV + attention: head-parallel. Core i owns heads {2i, 2i+1} for all
    8192 tokens (weight column split of Wq/Wk/Wv).
  - MLP + residual: token-parallel. Core i owns flat token rows
    [1024*i, 1024*(i+1)).
  - Bridge: one AllToAll of attn_out^T (bf16, 2 MB/core) reshards
    head-major columns -> token-major rows.

On-device layouts are transposed ("^T" = [feature, token]) so every matmul
contracts along SBUF partitions. Softmax denominators ride as a ones-column
prepended to V (V_aug, ones col first so the denominator lands on PSUM
partition 0); normalization uses a PE ones-broadcast matmul. All matmuls
run in bf16 with fp32 PSUM accumulation; the x-residual path stays fp32.
"""
import sys

sys.path.insert(0, "/opt/trn_rl_repo")

import numpy as np
import ml_dtypes

import concourse.bass as bass  # noqa: F401
import concourse.mybir as mybir
import concourse.tile as tile
from concourse import bacc
from concourse.bass_utils import run_bass_kernel_spmd
from concourse.masks import make_identity

FP32 = mybir.dt.float32
BF16 = mybir.dt.bfloat16
AF = mybir.ActivationFunctionType
ALU = mybir.AluOpType

N_CORES = 8
B, T, H = 4, 2048, 1024
NH, HD = 16, 64
FF = 4 * H
NTOK = B * T               # 8192 flat tokens
HPC = NH // N_CORES        # 2 heads per core
DPC = HPC * HD             # 128 head-dim rows per core
SCALE = 1.0 / float(H) ** 0.5

CH = 512                   # token chunk (matmul free dim)
KB = 128                   # k-block inside attention
QC = 512                   # attention q chunk
HK = H // 128              # 8 k-tiles over hidden dim
FK = FF // 128             # 32 tiles over FF dim


def tpc_of(t):
    return B * t // N_CORES


def build_kernel(debug_t=None):
    """Build the 8-core Bass module. Returns finalized nc."""
    t = T if debug_t is None else debug_t
    nqc = t // QC
    nkb = t // KB
    ntok = B * t
    nch = ntok // CH
    tpc = tpc_of(t)
    assert QC <= tpc, "q-chunk must not span multiple MLP owner cores"

    nc = bacc.Bacc(num_devices=N_CORES)

    # ---- I/O ----
    xbf_in = nc.declare_dram_parameter("xbf", [ntok, H], BF16, isOutput=False)
    xf_in = nc.declare_dram_parameter("xf", [tpc, H], FP32, isOutput=False)
    wq_in = nc.declare_dram_parameter("wq", [H, DPC], BF16, isOutput=False)
    wk_in = nc.declare_dram_parameter("wk", [H, DPC], BF16, isOutput=False)
    wv_in = nc.declare_dram_parameter("wv", [H, DPC], BF16, isOutput=False)
    bqkv_in = nc.declare_dram_parameter("bqkv", [DPC, 3], FP32, isOutput=False)
    # w1 host-packed [FK, 128p, HK, 128f]: w1[m, p, hk, f] = W1[hk*128+p, m*128+f]
    w1_in = nc.declare_dram_parameter("w1", [FK, 128, HK * 128], BF16, isOutput=False)
    b1_in = nc.declare_dram_parameter("b1", [128, FK], FP32, isOutput=False)
    # w2 host-packed [HK, 128p, FK, 128f]: w2[mh, p, k, f] = W2[k*128+p, mh*128+f]
    w2_in = nc.declare_dram_parameter("w2", [HK, 128, FK * 128], BF16, isOutput=False)
    b2_in = nc.declare_dram_parameter("b2", [128, HK], FP32, isOutput=False)
    out_d = nc.declare_dram_parameter("out", [tpc, H], FP32, isOutput=True)

    with tile.TileContext(nc) as tc:
        with (
            tc.tile_pool(name="const", bufs=1) as cpool,
            tc.tile_pool(name="ps_mm", bufs=3, space="PSUM") as ps_mm,
            tc.tile_pool(name="ps_tr", bufs=2, space="PSUM") as ps_tr,
            tc.tile_pool(name="ps_misc", bufs=2, space="PSUM") as ps_misc,
            tc.tile_pool(name="ps_o", bufs=1, space="PSUM") as ps_o,
            tc.tile_pool(name="dram", bufs=1, space="DRAM") as dpool,
        ):
            # ---------- constants ----------
            ident_b = cpool.tile([128, 128], BF16, tag="idb")
            make_identity(nc, ident_b)
            ident_f = cpool.tile([128, 128], FP32, tag="idf")
            make_identity(nc, ident_f)
            ones_b = cpool.tile([128, HD], BF16, tag="ones")
            nc.vector.memset(ones_b[:], 1.0)
            # stacked 64x64 identities at partition bases 0 and 64 (for
            # per-head V transposes whose inputs sit at base 0 / 64)
            ident2 = cpool.tile([128, HD], BF16, tag="id2")
            nc.gpsimd.memset(ident2[:], 0.0)
            for half in range(2):
                nc.gpsimd.affine_select(
                    out=ident2[:],
                    in_=ident2[:],
                    compare_op=ALU.not_equal,
                    fill=1.0,
                    base=-(HD * half),
                    channel_multiplier=1,
                    pattern=[[-1, HD]],
                )
            # causal 0/1 masks for the 4 diagonal offsets (S^T layout
            # [k-part, q-free]): keep iff j - p - 128*d >= 0
            masks = []
            for d in range(QC // KB):
                m = cpool.tile([128, QC], BF16, tag=f"mask{d}")
                nc.gpsimd.memset(m[:], 1.0)
                nc.gpsimd.affine_select(
                    out=m[:],
                    in_=m[:],
                    compare_op=ALU.is_ge,
                    fill=0.0,
                    base=-(KB * d),
                    channel_multiplier=-1,
                    pattern=[[1, QC]],
                )
                masks.append(m)
            # biases
            bqkv_sb = cpool.tile([DPC, 3], FP32, tag="bqkv")
            nc.sync.dma_start(out=bqkv_sb[:], in_=bqkv_in[:])
            b1_sb = cpool.tile([128, FK], FP32, tag="b1")
            nc.sync.dma_start(out=b1_sb[:], in_=b1_in[:])
            b2_sb = cpool.tile([128, HK], FP32, tag="b2")
            nc.sync.dma_start(out=b2_sb[:], in_=b2_in[:])
            # QKV weights, k-tile major: [128, HK*DPC]
            wq_sb = cpool.tile([128, HK * DPC], BF16, tag="wq")
            wk_sb = cpool.tile([128, HK * DPC], BF16, tag="wk")
            wv_sb = cpool.tile([128, HK * DPC], BF16, tag="wv")
            for w_sb, w_in in ((wq_sb, wq_in), (wk_sb, wk_in), (wv_sb, wv_in)):
                for hk in range(HK):
                    nc.sync.dma_start(
                        out=w_sb[:, hk * DPC : (hk + 1) * DPC],
                        in_=w_in[hk * 128 : (hk + 1) * 128, :],
                    )

            # a2a buffers (bf16 attn^T, dest-core major)
            a2a_in = dpool.tile([N_CORES, DPC, tpc], BF16, tag="a2ai")
            a2a_out = dpool.tile([N_CORES, DPC, tpc], BF16, tag="a2ao")

            with tc.tile_pool(name="qkvt", bufs=1) as qkvt:
                # persistent per-core Q^T/K^T/V^T  [128 = 2 heads x 64, ntok]
                qt_sb = qkvt.tile([128, ntok], BF16, tag="qt")
                kt_sb = qkvt.tile([128, ntok], BF16, tag="kt")
                vt_sb = qkvt.tile([128, ntok], BF16, tag="vt")

                # ------- Phase A: QKV projections (all tokens, 2 heads) ---
                with (
                    tc.tile_pool(name="xload", bufs=4) as xload,
                    tc.tile_pool(name="xtp", bufs=2) as xtp,
                ):
                    for c in range(nch):
                        xt_c = xtp.tile([128, HK * CH], BF16, tag="xt")
                        for pt in range(CH // 128):
                            x_sb = xload.tile([128, H], BF16, tag="x")
                            nc.sync.dma_start(
                                out=x_sb[:],
                                in_=xbf_in[
                                    c * CH + pt * 128 : c * CH + (pt + 1) * 128, :
                                ],
                            )
                            for hk in range(HK):
                                trp = ps_tr.tile([128, 128], BF16, tag="tr")
                                nc.tensor.transpose(
                                    trp[:],
                                    x_sb[:, hk * 128 : (hk + 1) * 128],
                                    ident_b[:],
                                )
                                nc.vector.tensor_copy(
                                    xt_c[
                                        :,
                                        hk * CH + pt * 128 : hk * CH + (pt + 1) * 128,
                                    ],
                                    trp[:],
                                )
                        for w_sb, dst, bi in (
                            (wq_sb, qt_sb, 0),
                            (wk_sb, kt_sb, 1),
                            (wv_sb, vt_sb, 2),
                        ):
                            ps = ps_mm.tile([128, CH], FP32, tag="mm")
                            for hk in range(HK):
                                nc.tensor.matmul(
                                    ps[:],
                                    w_sb[:, hk * DPC : (hk + 1) * DPC],
                                    xt_c[:, hk * CH : (hk + 1) * CH],
                                    start=(hk == 0),
                                    stop=(hk == HK - 1),
                                )
                            nc.scalar.activation(
                                dst[:, c * CH : (c + 1) * CH],
                                ps[:],
                                AF.Identity,
                                bias=bqkv_sb[:, bi : bi + 1],
                                scale=1.0,
                            )

                # ------- Phase B: attention per (b, head) -----------------
                with (
                    tc.tile_pool(name="vaug", bufs=2) as vaug,
                    tc.tile_pool(name="esb", bufs=4) as esb,
                    tc.tile_pool(name="norm", bufs=4) as norm,
                ):
                    for b in range(B):
                        tok0 = b * t
                        # V_aug [128 k, nkb*130]: per k-block, per head:
                        # ones col then 64 V cols (65 per head)
                        va = vaug.tile([128, nkb * 130], BF16, tag="va")
                        nc.vector.memset(va[:], 1.0)
                        for kb in range(nkb):
                            for h in range(2):
                                c0 = kb * 130 + h * 65
                                trp = ps_tr.tile([128, 128], BF16, tag="tr")
                                nc.tensor.transpose(
                                    trp[0:128, 0:HD],
                                    vt_sb[
                                        h * HD : (h + 1) * HD,
                                        tok0 + kb * KB : tok0 + (kb + 1) * KB,
                                    ],
                                    ident2[h * HD : (h + 1) * HD, :],
                                )
                                nc.vector.tensor_copy(
                                    va[:, c0 : c0 + HD], trp[0:128, 0:HD]
                                )
                        for h in range(2):
                            hs = h * HD
                            for qc in range(nqc):
                                q0 = tok0 + qc * QC
                                kbs = (qc + 1) * (QC // KB)
                                ops = ps_o.tile([HD + 1, QC], FP32, tag="o")
                                for kb in range(kbs):
                                    sps = ps_mm.tile([128, QC], FP32, tag="mm")
                                    nc.tensor.matmul(
                                        sps[:],
                                        kt_sb[
                                            hs : hs + HD,
                                            tok0 + kb * KB : tok0 + (kb + 1) * KB,
                                        ],
                                        qt_sb[hs : hs + HD, q0 : q0 + QC],
                                        start=True,
                                        stop=True,
                                    )
                                    e_t = esb.tile([128, QC], BF16, tag="e")
                                    nc.scalar.activation(
                                        e_t[:], sps[:], AF.Exp, scale=SCALE
                                    )
                                    d = kb - qc * (QC // KB)
                                    if d >= 0:
                                        nc.vector.tensor_mul(
                                            e_t[:], e_t[:], masks[d][:]
                                        )
                                    nc.tensor.matmul(
                                        ops[:],
                                        va[:, kb * 130 + h * 65 : kb * 130 + (h + 1) * 65],
                                        e_t[:],
                                        start=(kb == 0),
                                        stop=(kb == kbs - 1),
                                    )
                                # normalize: ops row 64 is the softmax
                                # denom, rows 0..63 are unnormalized attn^T
                                r_f = norm.tile([128, QC], FP32, tag="rf")
                                nc.vector.reciprocal(
                                    r_f[HD : HD + 1, :], ops[HD : HD + 1, :]
                                )
                                r_b = norm.tile([128, QC], BF16, tag="rb")
                                nc.vector.tensor_copy(
                                    r_b[HD : HD + 1, :], r_f[HD : HD + 1, :]
                                )
                                rbp = ps_misc.tile([HD, QC], FP32, tag="misc")
                                nc.tensor.matmul(
                                    rbp[:],
                                    ones_b[HD : HD + 1, :],
                                    r_b[HD : HD + 1, :],
                                    start=True,
                                    stop=True,
                                )
                                rb_sb = norm.tile([HD, QC], FP32, tag="rsb")
                                nc.vector.tensor_copy(rb_sb[:], rbp[:])
                                at_sb = norm.tile([HD, QC], BF16, tag="at")
                                nc.vector.tensor_mul(
                                    at_sb[:], ops[0:HD, :], rb_sb[:]
                                )
                                # stage into a2a input
                                gtok = b * t + qc * QC
                                j = gtok // tpc
                                off = gtok % tpc
                                nc.sync.dma_start(
                                    out=a2a_in[j, hs : hs + HD, off : off + QC],
                                    in_=at_sb[:],
                                )

            # ------- Phase C: AllToAll (head-major -> token-major) --------
            nc.gpsimd.collective_compute(
                "AllToAll",
                ALU.bypass,
                replica_groups=[list(range(N_CORES))],
                ins=[a2a_in.opt()],
                outs=[a2a_out.opt()],
            )

            # ------- Phase D: residual + MLP on own token chunk -----------
            with (
                tc.tile_pool(name="hts", bufs=1) as hts,
                tc.tile_pool(name="w1l", bufs=3) as w1l,
                tc.tile_pool(name="w2l", bufs=2) as w2l,
                tc.tile_pool(name="mload", bufs=2) as mload,
                tc.tile_pool(name="asb", bufs=1) as asb,
                tc.tile_pool(name="outs", bufs=4) as outs,
            ):
                htf = hts.tile([128, HK * tpc], FP32, tag="htf")
                htb = hts.tile([128, HK * tpc], BF16, tag="htb")
                # h^T = x^T (fp32, own rows) + attn^T
                for pt in range(tpc // 128):
                    xf_sb = mload.tile([128, H], FP32, tag="xf")
                    nc.sync.dma_start(
                        out=xf_sb[:], in_=xf_in[pt * 128 : (pt + 1) * 128, :]
                    )
                    for hk in range(HK):
                        trp = ps_misc.tile([128, 128], FP32, tag="misc")
                        nc.tensor.transpose(
                            trp[:],
                            xf_sb[:, hk * 128 : (hk + 1) * 128],
                            ident_f[:],
                        )
                        nc.vector.tensor_copy(
                            htf[:, hk * tpc + pt * 128 : hk * tpc + (pt + 1) * 128],
                            trp[:],
                        )
                for hk in range(HK):
                    atl = mload.tile([128, tpc], BF16, tag="atl")
                    nc.sync.dma_start(out=atl[:], in_=a2a_out[hk, :, :])
                    atf = mload.tile([128, tpc], FP32, tag="atf")
                    nc.vector.tensor_copy(atf[:], atl[:])
                    nc.vector.tensor_add(
                        htf[:, hk * tpc : (hk + 1) * tpc],
                        htf[:, hk * tpc : (hk + 1) * tpc],
                        atf[:],
                    )
                    nc.vector.tensor_copy(
                        htb[:, hk * tpc : (hk + 1) * tpc],
                        htf[:, hk * tpc : (hk + 1) * tpc],
                    )

                for cc in range(tpc // CH):
                    a_t = asb.tile([128, FK * CH], BF16, tag="a")
                    for m in range(FK):
                        w1_sb = w1l.tile([128, HK * 128], BF16, tag="w1")
                        nc.sync.dma_start(out=w1_sb[:], in_=w1_in[m, :, :])
                        aps = ps_mm.tile([128, CH], FP32, tag="mm")
                        for hk in range(HK):
                            nc.tensor.matmul(
                                aps[:],
                                w1_sb[:, hk * 128 : (hk + 1) * 128],
                                htb[
                                    :, hk * tpc + cc * CH : hk * tpc + (cc + 1) * CH
                                ],
                                start=(hk == 0),
                                stop=(hk == HK - 1),
                            )
                        nc.scalar.activation(
                            a_t[:, m * CH : (m + 1) * CH],
                            aps[:],
                            AF.Gelu_apprx_tanh,
                            bias=b1_sb[:, m : m + 1],
                            scale=1.0,
                        )
                    for mh in range(HK):
                        w2_sb = w2l.tile([128, FK * 128], BF16, tag="w2")
                        nc.sync.dma_start(out=w2_sb[:], in_=w2_in[mh, :, :])
                        yps = ps_mm.tile([128, CH], FP32, tag="mm")
                        for k in range(FK):
                            nc.tensor.matmul(
                                yps[:],
                                w2_sb[:, k * 128 : (k + 1) * 128],
                                a_t[:, k * CH : (k + 1) * CH],
                                start=(k == 0),
                                stop=(k == FK - 1),
                            )
                        # out^T = (y + b2) + h^T
                        otf = outs.tile([128, CH], FP32, tag="ot")
                        nc.vector.scalar_tensor_tensor(
                            out=otf[:],
                            in0=yps[:],
                            scalar=b2_sb[:, mh : mh + 1],
                            in1=htf[
                                :, mh * tpc + cc * CH : mh * tpc + (cc + 1) * CH
                            ],
                            op0=ALU.add,
                            op1=ALU.add,
                        )
                        for tt in range(CH // 128):
                            trp = ps_misc.tile([128, 128], FP32, tag="misc")
                            nc.tensor.transpose(
                                trp[:],
                                otf[:, tt * 128 : (tt + 1) * 128],
                                ident_f[:],
                            )
                            o_sb = outs.tile([128, 128], FP32, tag="osb")
                            nc.vector.tensor_copy(o_sb[:], trp[:])
                            r0 = cc * CH + tt * 128
                            nc.sync.dma_start(
                                out=out_d[r0 : r0 + 128, mh * 128 : (mh + 1) * 128],
                                in_=o_sb[:],
                            )

    nc.finalize()
    return nc


_NC_CACHE = {}


def _get_nc(debug_t=None):
    if debug_t not in _NC_CACHE:
        _NC_CACHE[debug_t] = build_kernel(debug_t)
    return _NC_CACHE[debug_t]


def kernel(x, Wq, bq, Wk, bk, Wv, bv, W1, b1, W2, b2, debug_t=None, _trace=False):
    t = T if debug_t is None else debug_t
    ntok = B * t
    tpc = tpc_of(t)

    x = np.asarray(x, dtype=np.float32).reshape(ntok, H)
    xbf = x.astype(ml_dtypes.bfloat16)
    w1p = np.ascontiguousarray(
        np.asarray(W1, np.float32)
        .reshape(HK, 128, FK, 128)
        .transpose(2, 1, 0, 3)
        .reshape(FK, 128, HK * 128)
    ).astype(ml_dtypes.bfloat16)
    w2p = np.ascontiguousarray(
        np.asarray(W2, np.float32)
        .reshape(FK, 128, HK, 128)
        .transpose(2, 1, 0, 3)
        .reshape(HK, 128, FK * 128)
    ).astype(ml_dtypes.bfloat16)
    b1p = np.ascontiguousarray(np.asarray(b1, np.float32).reshape(FK, 128).T)
    b2p = np.ascontiguousarray(np.asarray(b2, np.float32).reshape(HK, 128).T)

    in_maps = []
    for i in range(N_CORES):
        cols = slice(i * DPC, (i + 1) * DPC)
        bqkv = np.ascontiguousarray(
            np.stack(
                [
                    np.asarray(bq, np.float32)[cols],
                    np.asarray(bk, np.float32)[cols],
                    np.asarray(bv, np.float32)[cols],
                ],
                axis=1,
            )
        )
        in_maps.append(
            {
                "xbf": xbf,
                "xf": np.ascontiguousarray(x[i * tpc : (i + 1) * tpc]),
                "wq": np.ascontiguousarray(
                    np.asarray(Wq, np.float32)[:, cols]
                ).astype(ml_dtypes.bfloat16),
                "wk": np.ascontiguousarray(
                    np.asarray(Wk, np.float32)[:, cols]
                ).astype(ml_dtypes.bfloat16),
                "wv": np.ascontiguousarray(
                    np.asarray(Wv, np.float32)[:, cols]
                ).astype(ml_dtypes.bfloat16),
                "bqkv": bqkv,
                "w1": w1p,
                "b1": b1p,
                "w2": w2p,
                "b2": b2p,
            }
        )

    nc = _get_nc(debug_t)
    res = run_bass_kernel_spmd(
        nc, in_maps, core_ids=list(range(N_CORES)), trace=_trace
    )
    out = np.concatenate([res.results[i]["out"] for i in range(N_CORES)], axis=0)
    out = out.reshape(B, t, H)
    if _trace:
        return out, res
    return out


# revision 9
# speedup vs baseline: 2.0703x; 2.0703x over previous
"""Trainium2 Bass kernel for a dense transformer block (GPT style).

  out = h + gelu(h @ W1 + b1) @ W2 + b2,  h = x + attn(x)
  attn: 16-head causal self-attention, scale = 1/sqrt(H) (full hidden dim).

Shapes: x [4, 2048, 1024], NH=16, HD=64, FF=4096. 8 NeuronCores.

Distribution:
  - QKV + attention: head-parallel. Core i owns heads {2i, 2i+1} for all
    8192 tokens (weight column split of Wq/Wk/Wv).
  - MLP + residual: token-parallel. Core i owns flat token rows
    [1024*i, 1024*(i+1)).
  - Bridge: one AllToAll of attn_out^T (bf16, 2 MB/core) reshards
    head-major columns -> token-major rows.

On-device layouts are transposed ("^T" = [feature, token]) so every matmul
contracts along SBUF partitions. Softmax denominators ride as a ones-column
prepended to V (V_aug, ones col first so the denominator lands on PSUM
partition 0); normalization uses a PE ones-broadcast matmul. All matmuls
run in bf16 with fp32 PSUM accumulation; the x-residual path stays fp32.
"""
import sys

sys.path.insert(0, "/opt/trn_rl_repo")

import numpy as np
import ml_dtypes

import concourse.bass as bass  # noqa: F401
import concourse.mybir as mybir
import concourse.tile as tile
from concourse import bacc
from concourse.bass_utils import run_bass_kernel_spmd
from concourse.masks import make_identity

FP32 = mybir.dt.float32
BF16 = mybir.dt.bfloat16
AF = mybir.ActivationFunctionType
ALU = mybir.AluOpType

N_CORES = 8
B, T, H = 4, 2048, 1024
NH, HD = 16, 64
FF = 4 * H
NTOK = B * T               # 8192 flat tokens
HPC = NH // N_CORES        # 2 heads per core
DPC = HPC * HD             # 128 head-dim rows per core
SCALE = 1.0 / float(H) ** 0.5

CH = 512                   # token chunk (matmul free dim)
KB = 128                   # k-block inside attention
QC = 512                   # attention q chunk
HK = H // 128              # 8 k-tiles over hidden dim
FK = FF // 128             # 32 tiles over FF dim


def tpc_of(t):
    return B * t // N_CORES


def build_kernel(debug_t=None):
    """Build the 8-core Bass module. Returns finalized nc."""
    t = T if debug_t is None else debug_t
    nqc = t // QC
    nkb = t // KB
    ntok = B * t
    nch = ntok // CH
    tpc = tpc_of(t)
    assert QC <= tpc, "q-chunk must not span multiple MLP owner cores"

    nc = bacc.Bacc(num_devices=N_CORES)

    # ---- I/O ----
    xbf_in = nc.declare_dram_parameter("xbf", [ntok, H], BF16, isOutput=False)
    xf_in = nc.declare_dram_parameter("xf", [tpc, H], FP32, isOutput=False)
    wq_in = nc.declare_dram_parameter("wq", [H, DPC], BF16, isOutput=False)
    wk_in = nc.declare_dram_parameter("wk", [H, DPC], BF16, isOutput=False)
    wv_in = nc.declare_dram_parameter("wv", [H, DPC], BF16, isOutput=False)
    bqkv_in = nc.declare_dram_parameter("bqkv", [DPC, 3], FP32, isOutput=False)
    # w1 host-packed [FK, 128p, HK, 128f]: w1[m, p, hk, f] = W1[hk*128+p, m*128+f]
    w1_in = nc.declare_dram_parameter("w1", [FK, 128, HK * 128], BF16, isOutput=False)
    b1_in = nc.declare_dram_parameter("b1", [128, FK], FP32, isOutput=False)
    # w2 host-packed [HK, 128p, FK, 128f]: w2[mh, p, k, f] = W2[k*128+p, mh*128+f]
    w2_in = nc.declare_dram_parameter("w2", [HK, 128, FK * 128], BF16, isOutput=False)
    b2_in = nc.declare_dram_parameter("b2", [128, HK], FP32, isOutput=False)
    out_d = nc.declare_dram_parameter("out", [tpc, H], FP32, isOutput=True)

    with tile.TileContext(nc) as tc:
        with (
            tc.tile_pool(name="const", bufs=1) as cpool,
            tc.tile_pool(name="ps_mm", bufs=4, space="PSUM") as ps_mm,
            tc.tile_pool(name="ps_misc", bufs=2, space="PSUM") as ps_misc,
            tc.tile_pool(name="ps_o", bufs=2, space="PSUM") as ps_o,
            tc.tile_pool(name="dram", bufs=1, space="DRAM") as dpool,
        ):
            # ---------- constants ----------
            ident_b = cpool.tile([128, 128], BF16, tag="idb")
            make_identity(nc, ident_b)
            ident_f = cpool.tile([128, 128], FP32, tag="idf")
            make_identity(nc, ident_f)
            ones_b = cpool.tile([128, HD], BF16, tag="ones")
            nc.vector.memset(ones_b[:], 1.0)
            # stacked 64x64 identities at partition bases 0 and 64 (for
            # per-head V transposes whose inputs sit at base 0 / 64)
            ident2 = cpool.tile([128, HD], BF16, tag="id2")
            nc.gpsimd.memset(ident2[:], 0.0)
            for half in range(2):
                nc.gpsimd.affine_select(
                    out=ident2[:],
                    in_=ident2[:],
                    compare_op=ALU.not_equal,
                    fill=1.0,
                    base=-(HD * half),
                    channel_multiplier=1,
                    pattern=[[-1, HD]],
                )
            # causal 0/1 masks for the 4 diagonal offsets (S^T layout
            # [k-part, q-free]): keep iff j - p - 128*d >= 0
            masks = []
            for d in range(QC // KB):
                m = cpool.tile([128, QC], BF16, tag=f"mask{d}")
                nc.gpsimd.memset(m[:], 1.0)
                nc.gpsimd.affine_select(
                    out=m[:],
                    in_=m[:],
                    compare_op=ALU.is_ge,
                    fill=0.0,
                    base=-(KB * d),
                    channel_multiplier=-1,
                    pattern=[[1, QC]],
                )
                masks.append(m)
            # PE warm-up: ~10us of back-to-back matmuls so the HAM clock
            # gate opens (K=8/8) before real work lands
            for wi in range(20):
                wps = ps_mm.tile([128, QC], FP32, tag="mm")
                nc.tensor.matmul(
                    wps[:], ident_b[:], masks[0][:], start=True, stop=True
                )
            # biases
            bqkv_sb = cpool.tile([DPC, 3], FP32, tag="bqkv")
            nc.sync.dma_start(out=bqkv_sb[:], in_=bqkv_in[:])
            b1_sb = cpool.tile([128, FK], FP32, tag="b1")
            nc.sync.dma_start(out=b1_sb[:], in_=b1_in[:])
            b2_sb = cpool.tile([128, HK], FP32, tag="b2")
            nc.sync.dma_start(out=b2_sb[:], in_=b2_in[:])
            # QKV weights, k-tile major: [128, HK*DPC]
            wq_sb = cpool.tile([128, HK * DPC], BF16, tag="wq")
            wk_sb = cpool.tile([128, HK * DPC], BF16, tag="wk")
            wv_sb = cpool.tile([128, HK * DPC], BF16, tag="wv")
            for w_sb, w_in in ((wq_sb, wq_in), (wk_sb, wk_in), (wv_sb, wv_in)):
                for hk in range(HK):
                    nc.sync.dma_start(
                        out=w_sb[:, hk * DPC : (hk + 1) * DPC],
                        in_=w_in[hk * 128 : (hk + 1) * 128, :],
                    )

            # a2a buffers (bf16 attn^T, dest-core major)
            a2a_in = dpool.tile([N_CORES, DPC, tpc], BF16, tag="a2ai")
            a2a_out = dpool.tile([N_CORES, DPC, tpc], BF16, tag="a2ao")

            with tc.tile_pool(name="qkvt", bufs=1) as qkvt:
                # persistent per-core Q^T/K^T/V^T  [128 = 2 heads x 64, ntok]
                qt_sb = qkvt.tile([128, ntok], BF16, tag="qt")
                kt_sb = qkvt.tile([128, ntok], BF16, tag="kt")
                vt_sb = qkvt.tile([128, ntok], BF16, tag="vt")

                # ------- Phase A: QKV projections (all tokens, 2 heads) ---
                with (
                    tc.tile_pool(name="xload", bufs=4) as xload,
                    tc.tile_pool(name="xtp", bufs=2) as xtp,
                ):
                    for c in range(nch):
                        xt_c = xtp.tile([128, HK * CH], BF16, tag="xt")
                        for pt in range(CH // 128):
                            x_sb = xload.tile([128, H], BF16, tag="x")
                            nc.sync.dma_start(
                                out=x_sb[:],
                                in_=xbf_in[
                                    c * CH + pt * 128 : c * CH + (pt + 1) * 128, :
                                ],
                            )
                            for hk in range(HK):
                                trp = ps_misc.tile([128, 128], BF16, tag="misc")
                                nc.tensor.transpose(
                                    trp[:],
                                    x_sb[:, hk * 128 : (hk + 1) * 128],
                                    ident_b[:],
                                )
                                nc.vector.tensor_copy(
                                    xt_c[
                                        :,
                                        hk * CH + pt * 128 : hk * CH + (pt + 1) * 128,
                                    ],
                                    trp[:],
                                )
                        for w_sb, dst, bi in (
                            (wq_sb, qt_sb, 0),
                            (wk_sb, kt_sb, 1),
                            (wv_sb, vt_sb, 2),
                        ):
                            ps = ps_mm.tile([128, CH], FP32, tag="mm")
                            for hk in range(HK):
                                nc.tensor.matmul(
                                    ps[:],
                                    w_sb[:, hk * DPC : (hk + 1) * DPC],
                                    xt_c[:, hk * CH : (hk + 1) * CH],
                                    start=(hk == 0),
                                    stop=(hk == HK - 1),
                                )
                            nc.vector.tensor_scalar_add(
                                dst[:, c * CH : (c + 1) * CH],
                                ps[:],
                                bqkv_sb[:, bi : bi + 1],
                            )

                # ------- Phase B: attention per (b, head) -----------------
                with (
                    tc.tile_pool(name="vaug", bufs=2) as vaug,
                    tc.tile_pool(name="esb", bufs=4) as esb,
                    tc.tile_pool(name="norm", bufs=4) as norm,
                ):
                    for b in range(B):
                        tok0 = b * t
                        # V_aug [128 k, nkb*130]: per k-block, per head:
                        # ones col then 64 V cols (65 per head)
                        va = vaug.tile([128, nkb * 130], BF16, tag="va")
                        nc.vector.memset(va[:], 1.0)
                        for kb in range(nkb):
                            for h in range(2):
                                c0 = kb * 130 + h * 65
                                trp = ps_misc.tile([128, 128], BF16, tag="misc")
                                nc.tensor.transpose(
                                    trp[0:128, 0:HD],
                                    vt_sb[
                                        h * HD : (h + 1) * HD,
                                        tok0 + kb * KB : tok0 + (kb + 1) * KB,
                                    ],
                                    ident2[h * HD : (h + 1) * HD, :],
                                )
                                nc.vector.tensor_copy(
                                    va[:, c0 : c0 + HD], trp[0:128, 0:HD]
                                )
                        for qc in range(nqc):
                            q0 = tok0 + qc * QC
                            kbs = (qc + 1) * (QC // KB)
                            ops_a = ps_o.tile([HD + 1, QC], FP32, tag="o")
                            ops_b = ps_o.tile([HD + 1, QC], FP32, tag="o")
                            opss = [ops_a, ops_b]
                            for kb in range(kbs):
                                d = kb - qc * (QC // KB)
                                for h in range(2):
                                    hs = h * HD
                                    sps = ps_mm.tile([128, QC], FP32, tag="mm")
                                    nc.tensor.matmul(
                                        sps[:],
                                        kt_sb[
                                            hs : hs + HD,
                                            tok0 + kb * KB : tok0 + (kb + 1) * KB,
                                        ],
                                        qt_sb[hs : hs + HD, q0 : q0 + QC],
                                        start=True,
                                        stop=True,
                                    )
                                    e_t = esb.tile([128, QC], BF16, tag="e")
                                    nc.scalar.activation(
                                        e_t[:], sps[:], AF.Exp, scale=SCALE
                                    )
                                    if d >= 0:
                                        nc.vector.tensor_mul(
                                            e_t[:], e_t[:], masks[d][:]
                                        )
                                    nc.tensor.matmul(
                                        opss[h][:],
                                        va[
                                            :,
                                            kb * 130 + h * 65 : kb * 130 + (h + 1) * 65,
                                        ],
                                        e_t[:],
                                        start=(kb == 0),
                                        stop=(kb == kbs - 1),
                                    )
                            for h in range(2):
                                hs = h * HD
                                ops = opss[h]
                                # normalize: ops row 64 is the softmax
                                # denom, rows 0..63 are unnormalized attn^T
                                r_f = norm.tile([128, QC], FP32, tag="rf")
                                nc.vector.reciprocal(
                                    r_f[HD : HD + 1, :], ops[HD : HD + 1, :]
                                )
                                r_b = norm.tile([128, QC], BF16, tag="rb")
                                nc.vector.tensor_copy(
                                    r_b[HD : HD + 1, :], r_f[HD : HD + 1, :]
                                )
                                rbp = ps_misc.tile([HD, QC], FP32, tag="misc")
                                nc.tensor.matmul(
                                    rbp[:],
                                    ones_b[HD : HD + 1, :],
                                    r_b[HD : HD + 1, :],
                                    start=True,
                                    stop=True,
                                )
                                rb_sb = norm.tile([HD, QC], FP32, tag="rsb")
                                nc.vector.tensor_copy(rb_sb[:], rbp[:])
                                at_sb = norm.tile([HD, QC], BF16, tag="at")
                                nc.vector.tensor_mul(
                                    at_sb[:], ops[0:HD, :], rb_sb[:]
                                )
                                # stage into a2a input
                                gtok = b * t + qc * QC
                                j = gtok // tpc
                                off = gtok % tpc
                                nc.sync.dma_start(
                                    out=a2a_in[j, hs : hs + HD, off : off + QC],
                                    in_=at_sb[:],
                                )

            # ------- Phase C: AllToAll (head-major -> token-major) --------
            nc.gpsimd.collective_compute(
                "AllToAll",
                ALU.bypass,
                replica_groups=[list(range(N_CORES))],
                ins=[a2a_in.opt()],
                outs=[a2a_out.opt()],
            )

            # ------- Phase D: residual + MLP on own token chunk -----------
            with (
                tc.tile_pool(name="hts", bufs=1) as hts,
                tc.tile_pool(name="w1l", bufs=3) as w1l,
                tc.tile_pool(name="w2l", bufs=2) as w2l,
                tc.tile_pool(name="mload", bufs=2) as mload,
                tc.tile_pool(name="asb", bufs=1) as asb,
                tc.tile_pool(name="outs", bufs=4) as outs,
            ):
                htf = hts.tile([128, HK * tpc], FP32, tag="htf")
                htb = hts.tile([128, HK * tpc], BF16, tag="htb")
                # h^T = x^T (fp32, own rows) + attn^T
                for pt in range(tpc // 128):
                    xf_sb = mload.tile([128, H], FP32, tag="xf")
                    nc.sync.dma_start(
                        out=xf_sb[:], in_=xf_in[pt * 128 : (pt + 1) * 128, :]
                    )
                    for hk in range(HK):
                        trp = ps_misc.tile([128, 128], FP32, tag="misc")
                        nc.tensor.transpose(
                            trp[:],
                            xf_sb[:, hk * 128 : (hk + 1) * 128],
                            ident_f[:],
                        )
                        nc.vector.tensor_copy(
                            htf[:, hk * tpc + pt * 128 : hk * tpc + (pt + 1) * 128],
                            trp[:],
                        )
                for hk in range(HK):
                    atl = mload.tile([128, tpc], BF16, tag="atl")
                    nc.sync.dma_start(out=atl[:], in_=a2a_out[hk, :, :])
                    atf = mload.tile([128, tpc], FP32, tag="atf")
                    nc.vector.tensor_copy(atf[:], atl[:])
                    nc.vector.tensor_add(
                        htf[:, hk * tpc : (hk + 1) * tpc],
                        htf[:, hk * tpc : (hk + 1) * tpc],
                        atf[:],
                    )
                    nc.vector.tensor_copy(
                        htb[:, hk * tpc : (hk + 1) * tpc],
                        htf[:, hk * tpc : (hk + 1) * tpc],
                    )

                for cc in range(tpc // CH):
                    a_t = asb.tile([128, FK * CH], BF16, tag="a")
                    for m in range(FK):
                        w1_sb = w1l.tile([128, HK * 128], BF16, tag="w1")
                        nc.sync.dma_start(out=w1_sb[:], in_=w1_in[m, :, :])
                        aps = ps_mm.tile([128, CH], FP32, tag="mm")
                        for hk in range(HK):
                            nc.tensor.matmul(
                                aps[:],
                                w1_sb[:, hk * 128 : (hk + 1) * 128],
                                htb[
                                    :, hk * tpc + cc * CH : hk * tpc + (cc + 1) * CH
                                ],
                                start=(hk == 0),
                                stop=(hk == HK - 1),
                            )
                        nc.scalar.activation(
                            a_t[:, m * CH : (m + 1) * CH],
                            aps[:],
                            AF.Gelu_apprx_tanh,
                            bias=b1_sb[:, m : m + 1],
                            scale=1.0,
                        )
                    for mh in range(HK):
                        w2_sb = w2l.tile([128, FK * 128], BF16, tag="w2")
                        nc.sync.dma_start(out=w2_sb[:], in_=w2_in[mh, :, :])
                        yps = ps_mm.tile([128, CH], FP32, tag="mm")
                        for k in range(FK):
                            nc.tensor.matmul(
                                yps[:],
                                w2_sb[:, k * 128 : (k + 1) * 128],
                                a_t[:, k * CH : (k + 1) * CH],
                                start=(k == 0),
                                stop=(k == FK - 1),
                            )
                        # out^T = (y + b2) + h^T
                        otf = outs.tile([128, CH], FP32, tag="ot")
                        nc.vector.scalar_tensor_tensor(
                            out=otf[:],
                            in0=yps[:],
                            scalar=b2_sb[:, mh : mh + 1],
                            in1=htf[
                                :, mh * tpc + cc * CH : mh * tpc + (cc + 1) * CH
                            ],
                            op0=ALU.add,
                            op1=ALU.add,
                        )
                        for tt in range(CH // 128):
                            trp = ps_misc.tile([128, 128], FP32, tag="misc")
                            nc.tensor.transpose(
                                trp[:],
                                otf[:, tt * 128 : (tt + 1) * 128],
                                ident_f[:],
                            )
                            o_sb = outs.tile([128, 128], FP32, tag="osb")
                            nc.vector.tensor_copy(o_sb[:], trp[:])
                            r0 = cc * CH + tt * 128
                            nc.sync.dma_start(
                                out=out_d[r0 : r0 + 128, mh * 128 : (mh + 1) * 128],
                                in_=o_sb[:],
                            )

    nc.finalize()
    return nc


_NC_CACHE = {}


def _get_nc(debug_t=None):
    if debug_t not in _NC_CACHE:
        _NC_CACHE[debug_t] = build_kernel(debug_t)
    return _NC_CACHE[debug_t]


def kernel(x, Wq, bq, Wk, bk, Wv, bv, W1, b1, W2, b2, debug_t=None, _trace=False):
    t = T if debug_t is None else debug_t
    ntok = B * t
    tpc = tpc_of(t)

    x = np.asarray(x, dtype=np.float32).reshape(ntok, H)
    xbf = x.astype(ml_dtypes.bfloat16)
    w1p = np.ascontiguousarray(
        np.asarray(W1, np.float32)
        .reshape(HK, 128, FK, 128)
        .transpose(2, 1, 0, 3)
        .reshape(FK, 128, HK * 128)
    ).astype(ml_dtypes.bfloat16)
    w2p = np.ascontiguousarray(
        np.asarray(W2, np.float32)
        .reshape(FK, 128, HK, 128)
        .transpose(2, 1, 0, 3)
        .reshape(HK, 128, FK * 128)
    ).astype(ml_dtypes.bfloat16)
    b1p = np.ascontiguousarray(np.asarray(b1, np.float32).reshape(FK, 128).T)
    b2p = np.ascontiguousarray(np.asarray(b2, np.float32).reshape(HK, 128).T)

    in_maps = []
    for i in range(N_CORES):
        cols = slice(i * DPC, (i + 1) * DPC)
        bqkv = np.ascontiguousarray(
            np.stack(
                [
                    np.asarray(bq, np.float32)[cols],
                    np.asarray(bk, np.float32)[cols],
                    np.asarray(bv, np.float32)[cols],
                ],
                axis=1,
            )
        )
        in_maps.append(
            {
                "xbf": xbf,
                "xf": np.ascontiguousarray(x[i * tpc : (i + 1) * tpc]),
                "wq": np.ascontiguousarray(
                    np.asarray(Wq, np.float32)[:, cols]
                ).astype(ml_dtypes.bfloat16),
                "wk": np.ascontiguousarray(
                    np.asarray(Wk, np.float32)[:, cols]
                ).astype(ml_dtypes.bfloat16),
                "wv": np.ascontiguousarray(
                    np.asarray(Wv, np.float32)[:, cols]
                ).astype(ml_dtypes.bfloat16),
                "bqkv": bqkv,
                "w1": w1p,
                "b1": b1p,
                "w2": w2p,
                "b2": b2p,
            }
        )

    nc = _get_nc(debug_t)
    res = run_bass_kernel_spmd(
        nc, in_maps, core_ids=list(range(N_CORES)), trace=_trace
    )
    out = np.concatenate([res.results[i]["out"] for i in range(N_CORES)], axis=0)
    out = out.reshape(B, t, H)
    if _trace:
        return out, res
    return out
